# revision 97
# baseline (speedup 1.0000x reference)
"""Mistral attention (B=2, S=2048, H=4096, 32 q heads / 8 kv heads, rope) on
8 Trainium2 NeuronCores — compensated-fp8 DoubleRow design.

Sharding: DP=2 over batch x TP=4 over head groups (core c: batch c//4,
head group c%4). Each core computes a partial o_proj output [2048, 4096]
in bf16; host sums 4 TP partials per batch and descales by 1/32.

Precision strategy (the 2e-2 relmax gate needs ~bf16 quality everywhere;
raw fp8 is too noisy):
  - The three big GEMMs (Q/K proj, V proj, o_proj) run as 3-term hi/lo
    compensated fp8e4m3 DoubleRow: x = xh + xl, w = wh + wl, computing
    xh@wh + xl@wh + xh@wl (the dropped xl@wl term is ~2^-8). Cost is
    0.75x of bf16 (DoubleRow = 0.5 cyc/row packing 2 k-tiles/instr).
    x/w splits are host-side; the y split for o_proj is ACT cast + DVE sub.
  - The attention core (scores, exp, P, PV, normalizer) is bf16: P in
    fp8 fails the gate (measured), and bf16 exp needs no bias machinery.
  - RoPE: ACT copies PSUM->bf16, then four DVE half-ops (bf16 2x mode)
    with sign-arranged sin tables (1/32 weight descale folded in).
  - Causal mask: multiplicative 0/1 on the bf16 P tiles (DVE), with
    per-slot column narrowing on the diagonal strip; scores and PV are
    narrowed to the causal hull per key tile.
  - Softmax denominator: P column-sums accumulate on DVE (bf16) per
    128-key slot; the cross-partition reduce runs on the idle gpsimd
    engine (partition_all_reduce, fp32 internally) -- the PE does no
    normalizer work at all.
  - Scheduling: pass B (xl@wh) of each projection head is deferred two
    heads so the xl DMA is off the startup critical path; o_proj work
    for strip qj is deferred into strip qj+1's attention (4 tiles per
    head, spread evenly over the kp sites) to fill exp-latency PE
    stalls; strip 3's V-projection is deferred into attention strip 0
    (which has no o_proj filler yet), re-fetching that x strip; each
    head's softmax tail is deferred past the next head's first scores,
    and each strip's last-head tail runs immediately, column-chunked,
    so the next strip's o_proj pops (and the final drain) aren't
    blocked on its ~3.5us serial reduce/normalize chain.
"""
import sys

if "/opt/trn_rl_repo" not in sys.path:
    sys.path.insert(0, "/opt/trn_rl_repo")

import numpy as np
import ml_dtypes

BF16 = ml_dtypes.bfloat16
FP8 = ml_dtypes.float8_e4m3

S = 2048
H = 4096
D = 128
TP = 4
DP = 2
NHC = 8
NKVC = 2
DQ = NHC * D
DKV = NKVC * D
KO = H // 128
NT = S // 128
NSTRIP = S // 512
SCALE = 1.0 / np.sqrt(D)
ROPE_THETA = 10000.0
WS = 32.0

_CACHE = {}


def _build_nc():
    import concourse.mybir as mybir
    import concourse.tile as tile
    from concourse import bacc

    DT = mybir.dt
    DR = mybir.MatmulPerfMode.DoubleRow
    nc = bacc.Bacc(None, target_bir_lowering=False)

    xh8 = nc.dram_tensor("xh8", [H, S], DT.float8e4, kind="ExternalInput")
    xl8 = nc.dram_tensor("xl8", [H, S], DT.float8e4, kind="ExternalInput")
    # wq hi+lo packed per head [128, NHC, 2, KO, 128]: one 1MB DMA per head
    # (bigger DMAs amortize the fixed per-DMA HWDGE/DGE overheads)
    wq8t = nc.dram_tensor("wq8t", [128, NHC, 2, KO, 128], DT.float8e4,
                          kind="ExternalInput")
    # wk/wv hi+lo packed [128, 2, KO, DKV]: one 2MB DMA each, 8KB
    # contiguous per partition (no sub-512B descriptor penalty)
    wk8t = nc.dram_tensor("wk8t", [128, 2, KO, DKV], DT.float8e4,
                          kind="ExternalInput")
    wv8t = nc.dram_tensor("wv8t", [128, 2, KO, DKV], DT.float8e4,
                          kind="ExternalInput")
    woh8 = nc.dram_tensor("woh8", [DQ, H], DT.float8e4, kind="ExternalInput")
    wol8 = nc.dram_tensor("wol8", [DQ, H], DT.float8e4, kind="ExternalInput")
    cosb = nc.dram_tensor("cosb", [D, S], DT.bfloat16, kind="ExternalInput")
    sin2b = nc.dram_tensor("sin2b", [D, S], DT.bfloat16, kind="ExternalInput")
    masks = nc.dram_tensor("masks", [128, 4, 512], DT.bfloat16,
                           kind="ExternalInput")
    outp = nc.dram_tensor("outp", [S, H], DT.bfloat16, kind="ExternalOutput")

    xh_r = xh8.rearrange("(ko p) t -> p ko t", p=128)
    xl_r = xl8.rearrange("(ko p) t -> p ko t", p=128)
    woh_r = woh8.rearrange("(h p) n -> p h n", p=128)
    wol_r = wol8.rearrange("(h p) n -> p h n", p=128)

    with tile.TileContext(nc) as tc:
        with tc.tile_pool(name="persist", bufs=1) as persist:
            qT = persist.tile([128, NHC, S], DT.bfloat16)
            kT = persist.tile([128, NKVC, S], DT.bfloat16)
            vsb = persist.tile([128, NT, DKV], DT.bfloat16)
            # wv survives into phase 2: strip 3's V-projection is deferred
            # there as PE filler for the latency-bound first attention strip
            wv_sb = persist.tile([128, 2, KO, DKV], DT.float8e4)
            # y hi/lo fp8, ring of 2 query strips for deferred o_proj
            yh8 = persist.tile([128, NHC, 2, 512], DT.float8e4)
            yl8 = persist.tile([128, NHC, 2, 512], DT.float8e4)
            # causal masks live in persist so their DMA can issue during
            # phase-1 strip 3 (hides the phase-2 warmup)
            mask_sb = persist.tile([128, 4, 512], DT.bfloat16)

            # ------------- Phase 1: projections + rope -------------
            with tc.tile_pool(name="p1c", bufs=1) as p1c, \
                 tc.tile_pool(name="p1w", bufs=3) as p1w, \
                 tc.tile_pool(name="p1x", bufs=2) as p1x, \
                 tc.tile_pool(name="p1t", bufs=2) as p1t, \
                 tc.tile_pool(name="psP", bufs=4, space="PSUM") as psPp, \
                 tc.tile_pool(name="psV", bufs=2, space="PSUM") as psVp:
                # DMA queue order follows first-use order: head 0 weights and
                # x strip 0 in quarter-chunks (PE starts as soon as the first
                # k-tiles land), heads 1-2 weights, then the rope tables
                # (consumed by DVE only -- can be late) and k/v weights
                # staggered between streamed wq heads.
                w0 = p1w.tile([128, 2, KO, 128], DT.float8e4, tag="w")
                nc.sync.dma_start(w0[:, 0, 0:KO // 2, :],
                                  wq8t[:, 0, 0, 0:KO // 2, :])
                xh0 = p1x.tile([128, KO, 512], DT.float8e4, tag="xh")
                nc.sync.dma_start(xh0[:, 0:KO // 4, :],
                                  xh_r[:, 0:KO // 4, 0:512])
                nc.sync.dma_start(w0[:, 0, KO // 2:, :],
                                  wq8t[:, 0, 0, KO // 2:, :])
                for q in range(1, 4):
                    nc.sync.dma_start(
                        xh0[:, q * KO // 4:(q + 1) * KO // 4, :],
                        xh_r[:, q * KO // 4:(q + 1) * KO // 4, 0:512])
                # pass C is deferred one head and pass B two heads, so the
                # hi weights stream a head ahead of the lo weights and xl is
                # not needed until ~head 2 -- the queue below follows that
                # first-use order exactly
                pre_w = {0: w0}
                wt1 = p1w.tile([128, 2, KO, 128], DT.float8e4, tag="w")
                nc.sync.dma_start(wt1[:, 0], wq8t[:, 1, 0])
                pre_w[1] = wt1
                nc.sync.dma_start(w0[:, 1, :, :], wq8t[:, 0, 1, :, :])
                wt2 = p1w.tile([128, 2, KO, 128], DT.float8e4, tag="w")
                nc.sync.dma_start(wt2[:, 0], wq8t[:, 2, 0])
                pre_w[2] = wt2
                nc.sync.dma_start(wt1[:, 1], wq8t[:, 1, 1])
                xl0 = p1x.tile([128, KO, 512], DT.float8e4, tag="xl")
                for q in range(4):
                    nc.sync.dma_start(
                        xl0[:, q * KO // 4:(q + 1) * KO // 4, :],
                        xl_r[:, q * KO // 4:(q + 1) * KO // 4, 0:512])
                # cos/sin are consumed only by DVE rope ops which can lag;
                # their DMAs are issued mid-strip-0 (h==3) to keep the early
                # wq weight stream unblocked
                cos_sb = p1c.tile([D, S], DT.bfloat16)
                sin_sb = p1c.tile([D, S], DT.bfloat16)
                wk_sb = p1c.tile([128, 2, KO, DKV], DT.float8e4)
                # wk/wv DMAs are emitted inside the first strip's head loop so
                # they don't delay the streamed wq head slices

                rope_tail = [None]

                def _do_rope(h, qb, t0):
                    # all-bf16 DVE half-ops (2x mode); sin table rows are
                    # sign-arranged so both SBUF inputs share a base partition
                    av = p1t.tile([128, 512], DT.bfloat16, tag="av")
                    nc.vector.tensor_mul(av[:], qb[:], cos_sb[:, t0:t0 + 512])
                    bv = p1t.tile([128, 512], DT.bfloat16, tag="bv")
                    nc.vector.tensor_mul(bv[0:64, :], qb[64:128, :],
                                         sin_sb[64:128, t0:t0 + 512])
                    nc.vector.tensor_mul(bv[64:128, :], qb[0:64, :],
                                         sin_sb[0:64, t0:t0 + 512])
                    if h < NHC:
                        dst = qT[:, h, t0:t0 + 512]
                    else:
                        dst = kT[:, h - NHC, t0:t0 + 512]
                    nc.vector.tensor_add(dst, av[:], bv[:])

                for ts in range(NSTRIP):
                    t0 = ts * 512
                    if ts == 0:
                        xh, xl = xh0, xl0
                    else:
                        xh = p1x.tile([128, KO, 512], DT.float8e4, tag="xh")
                        nc.sync.dma_start(xh[:], xh_r[:, :, t0:t0 + 512])
                        xl = p1x.tile([128, KO, 512], DT.float8e4, tag="xl")
                        nc.sync.dma_start(xl[:], xl_r[:, :, t0:t0 + 512])
                    if ts == 3:
                        nc.sync.dma_start(mask_sb[:], masks[:])
                    pend = []
                    pend_c = []
                    w_tiles = {}

                    def emit_c(pend_c=pend_c, xh=xh):
                        # deferred pass C (xh @ wl) of head h-1
                        ps2, wls2 = pend_c.pop(0)
                        NP = KO // 2
                        for kp in range(NP):
                            nc.tensor.matmul(ps2[:],
                                             wls2[:, 2 * kp:2 * kp + 2, :],
                                             xh[:, 2 * kp:2 * kp + 2, :],
                                             start=False, stop=False,
                                             perf_mode=DR)

                    def emit_b(pend=pend, xl=xl, t0=t0):
                        # deferred pass B (xl @ wh) of head h-2, plus its
                        # PSUM->bf16 copy and rope
                        h2, ps2, whs2 = pend.pop(0)
                        NP = KO // 2
                        for kp in range(NP):
                            nc.tensor.matmul(ps2[:],
                                             whs2[:, 2 * kp:2 * kp + 2, :],
                                             xl[:, 2 * kp:2 * kp + 2, :],
                                             start=False, stop=(kp == NP - 1),
                                             perf_mode=DR)
                        qb = p1t.tile([128, 512], DT.bfloat16, tag="qb",
                                      bufs=4)
                        nc.scalar.copy(qb[:], ps2[:])

                        def rope(h=h2, qb=qb, t0=t0):
                            _do_rope(h, qb, t0)
                        if rope_tail[0] is not None:
                            rope_tail[0]()
                        rope_tail[0] = rope

                    for h in range(NHC + NKVC):
                        ps = psPp.tile([128, 512], DT.float32, tag="proj")
                        if h < NHC:
                            if ts == 0 and h in pre_w:
                                wt = pre_w[h]
                            else:
                                wt = p1w.tile([128, 2, KO, 128], DT.float8e4,
                                              tag="w")
                                nc.sync.dma_start(wt[:, 0], wq8t[:, h, 0])
                            w_tiles[h] = wt
                            whs = wt[:, 0, :, :]
                            wls = wt[:, 1, :, :]
                        else:
                            kv = h - NHC
                            whs = wk_sb[:, 0, :, kv * 128:(kv + 1) * 128]
                            wls = wk_sb[:, 1, :, kv * 128:(kv + 1) * 128]
                        # lo weights of the previous head (its pass C runs
                        # this iteration) -- one head behind the hi stream
                        if (h - 1) in w_tiles and not (ts == 0 and h <= 2):
                            nc.sync.dma_start(w_tiles[h - 1][:, 1],
                                              wq8t[:, h - 1, 1])
                        if ts == 0 and h == 3:
                            # rope tables: must be emitted before the first
                            # rope DVE ops (at this iteration's deferred
                            # pass B) so the dependency is tracked, but
                            # after the JIT weight DMAs above
                            nc.sync.dma_start(cos_sb[:], cosb[:])
                            nc.sync.dma_start(sin_sb[:], sin2b[:])
                        if ts == 0 and h == 6:
                            nc.sync.dma_start(wk_sb[:, 0], wk8t[:, 0])
                        if ts == 0 and h == 7:
                            nc.sync.dma_start(wk_sb[:, 1], wk8t[:, 1])
                        if ts == 0 and h == 8:
                            nc.sync.dma_start(wv_sb[:, 0], wv8t[:, 0])
                        if ts == 0 and h == 9:
                            nc.sync.dma_start(wv_sb[:, 1], wv8t[:, 1])
                        NP = KO // 2
                        for kp in range(NP):  # pass A: xh @ wh
                            nc.tensor.matmul(ps[:], whs[:, 2 * kp:2 * kp + 2, :],
                                             xh[:, 2 * kp:2 * kp + 2, :],
                                             start=(kp == 0), stop=False,
                                             perf_mode=DR)
                        if pend_c:
                            emit_c()  # pass C of head h-1
                        pend_c.append((ps, wls))
                        pend.append((h, ps, whs))
                        if len(pend) > 2:
                            emit_b()
                    while pend_c:
                        emit_c()
                    while pend:
                        emit_b()
                    # V (natural layout), 3-term comp, descaled 1/32 -> bf16
                    if ts == 3:
                        # V-proj for strip 3 is deferred into phase 2
                        if rope_tail[0] is not None:
                            rope_tail[0]()
                            rope_tail[0] = None
                        continue
                    for mt in range(4):
                        psv = psVp.tile([128, DKV], DT.float32, tag="pv")
                        NP = KO // 2
                        for kp in range(NP):
                            nc.tensor.matmul(psv[:], xh[:, 2 * kp:2 * kp + 2,
                                                        mt * 128:(mt + 1) * 128],
                                             wv_sb[:, 0, 2 * kp:2 * kp + 2, :],
                                             start=(kp == 0), stop=False,
                                             perf_mode=DR)
                        for kp in range(NP):
                            nc.tensor.matmul(psv[:], xl[:, 2 * kp:2 * kp + 2,
                                                        mt * 128:(mt + 1) * 128],
                                             wv_sb[:, 0, 2 * kp:2 * kp + 2, :],
                                             start=False, stop=False,
                                             perf_mode=DR)
                        for kp in range(NP):
                            nc.tensor.matmul(psv[:], xh[:, 2 * kp:2 * kp + 2,
                                                        mt * 128:(mt + 1) * 128],
                                             wv_sb[:, 1, 2 * kp:2 * kp + 2, :],
                                             start=False, stop=(kp == NP - 1),
                                             perf_mode=DR)
                        if mt == 0 and rope_tail[0] is not None:
                            rope_tail[0]()
                            rope_tail[0] = None
                        nc.vector.tensor_scalar_mul(vsb[:, ts * 4 + mt, :],
                                                    psv[:], 1.0 / WS)

            # ------------- Phases 2+3 interleaved, qj-major -------------
            with tc.tile_pool(name="p3w", bufs=1) as p3w:
                wo_sb = p3w.tile([128, NHC, 2, H], DT.float8e4)
                with tc.tile_pool(name="p2c", bufs=1) as p2c, \
                     tc.tile_pool(name="p2pt", bufs=7) as p2pt, \
                     tc.tile_pool(name="p2r", bufs=3) as p2r, \
                     tc.tile_pool(name="p2y", bufs=3) as p2y, \
                     tc.tile_pool(name="p2z", bufs=3) as p2z, \
                     tc.tile_pool(name="p3o", bufs=3) as p3o, \
                     tc.tile_pool(name="psS", bufs=4, space="PSUM") as psSp, \
                     tc.tile_pool(name="psO", bufs=3, space="PSUM") as psOp, \
                     tc.tile_pool(name="psO3", bufs=1, space="PSUM") as psO3p:
                    # x strip 3 re-fetched for the deferred V-projection
                    xv_h = p2c.tile([128, KO, 512], DT.float8e4)
                    xv_l = p2c.tile([128, KO, 512], DT.float8e4)
                    nc.sync.dma_start(xv_h[:], xh_r[:, :, 1536:2048])
                    nc.sync.dma_start(xv_l[:], xl_r[:, :, 1536:2048])
                    nc.sync.dma_start(wo_sb[:, :, 0, :], woh_r[:])
                    nc.sync.dma_start(wo_sb[:, :, 1, :], wol_r[:])

                    vproj_work = [0, 1, 2, 3]

                    def emit_vproj(mt):
                        # psO3's bank is idle during attention strip 0 (no
                        # o_proj pops yet) -- using it keeps the V units out
                        # of the ps_o/tail rotation
                        psv = psO3p.tile([128, 512], DT.float32, tag="p3")
                        NPk = KO // 2
                        for kp in range(NPk):
                            nc.tensor.matmul(
                                psv[:, 0:DKV],
                                xv_h[:, 2 * kp:2 * kp + 2,
                                     mt * 128:(mt + 1) * 128],
                                wv_sb[:, 0, 2 * kp:2 * kp + 2, :],
                                start=(kp == 0), stop=False, perf_mode=DR)
                        for kp in range(NPk):
                            nc.tensor.matmul(
                                psv[:, 0:DKV],
                                xv_l[:, 2 * kp:2 * kp + 2,
                                     mt * 128:(mt + 1) * 128],
                                wv_sb[:, 0, 2 * kp:2 * kp + 2, :],
                                start=False, stop=False, perf_mode=DR)
                        for kp in range(NPk):
                            nc.tensor.matmul(
                                psv[:, 0:DKV],
                                xv_h[:, 2 * kp:2 * kp + 2,
                                     mt * 128:(mt + 1) * 128],
                                wv_sb[:, 1, 2 * kp:2 * kp + 2, :],
                                start=False, stop=(kp == NPk - 1),
                                perf_mode=DR)
                        nc.vector.tensor_scalar_mul(vsb[:, 12 + mt, :],
                                                    psv[:, 0:DKV], 1.0 / WS)

                    oproj_work = []

                    def emit_oproj(qj, tt, nt, copy_eng="act", use_ps_s=False):
                        n0 = nt * 512
                        rj = qj % 2
                        tq = tt - 4 * qj
                        if use_ps_s:
                            ps3 = psSp.tile([128, 512], DT.float32, tag="s")
                        else:
                            ps3 = psO3p.tile([128, 512], DT.float32, tag="p3")
                        NPH = NHC // 2
                        for hp in range(NPH):  # yh @ woh
                            nc.tensor.matmul(
                                ps3[:],
                                yh8[:, 2 * hp:2 * hp + 2, rj,
                                    tq * 128:(tq + 1) * 128],
                                wo_sb[:, 2 * hp:2 * hp + 2, 0, n0:n0 + 512],
                                start=(hp == 0), stop=False, perf_mode=DR)
                        for hp in range(NPH):  # yl @ woh
                            nc.tensor.matmul(
                                ps3[:],
                                yl8[:, 2 * hp:2 * hp + 2, rj,
                                    tq * 128:(tq + 1) * 128],
                                wo_sb[:, 2 * hp:2 * hp + 2, 0, n0:n0 + 512],
                                start=False, stop=False, perf_mode=DR)
                        for hp in range(NPH):  # yh @ wol
                            nc.tensor.matmul(
                                ps3[:],
                                yh8[:, 2 * hp:2 * hp + 2, rj,
                                    tq * 128:(tq + 1) * 128],
                                wo_sb[:, 2 * hp:2 * hp + 2, 1, n0:n0 + 512],
                                start=False, stop=(hp == NPH - 1), perf_mode=DR)
                        ot = p3o.tile([128, 512], DT.bfloat16, tag="ot")
                        if copy_eng == "dve":
                            nc.vector.tensor_copy(ot[:], ps3[:])
                        else:
                            nc.scalar.copy(ot[:], ps3[:])
                        nc.sync.dma_start(
                            outp[tt * 128:(tt + 1) * 128, n0:n0 + 512], ot[:])

                    popeng = [0]

                    def pop_oproj(n, copy_eng=None, use_ps_s=False):
                        for _ in range(min(n, len(oproj_work))):
                            qjw, tt, nt = oproj_work.pop(0)
                            # alternate the PSUM->SBUF copy between Act and
                            # DVE: during attention strips Act paces the
                            # exp chain, so don't pile copies onto it
                            eng = copy_eng or ("act" if popeng[0] % 2 else
                                               "dve")
                            popeng[0] += 1
                            emit_oproj(qjw, tt, nt, eng, use_ps_s)

                    head_tail = [None]

                    for qj in range(4):
                        q0 = qj * 512
                        nkp = (4 * qj + 4) // 2
                        for h in range(NHC):
                            kv = h // (NHC // NKVC)
                            ps_o = psOp.tile([128, 512], DT.float32, tag="o")
                            # softmax-denominator partial sums, accumulated on
                            # DVE (bf16) instead of PE ones-matmuls; one final
                            # ones-matmul in the tail does the exact fp32
                            # cross-partition reduce
                            zacc = p2z.tile([128, 512], DT.bfloat16, tag="z")
                            pts = {}

                            def emit_scores(kp, qj=qj, h=h, kv=kv, q0=q0,
                                            pts=pts, zacc=zacc):
                                # one PSUM tile + exp per 128-key slot:
                                # 4-deep psS rotation rides out ACT lag, and
                                # each PV unblocks on a smaller exp
                                c0 = 256 if 2 * kp == 4 * qj + 2 else 0
                                for i in range(2):
                                    ki = 2 * kp + i
                                    k0 = ki * 128
                                    ci = max(c0, (ki - 4 * qj) * 128)
                                    ps_s = psSp.tile([128, 512], DT.float32,
                                                     tag="s")
                                    nc.tensor.matmul(
                                        ps_s[:, ci:],
                                        kT[:, kv, k0:k0 + 128],
                                        qT[:, h, q0 + ci:q0 + 512],
                                        start=True, stop=True)
                                    pt = p2pt.tile([128, 512], DT.bfloat16,
                                                   tag="pt")
                                    nc.scalar.activation(
                                        pt[:, ci:], ps_s[:, ci:],
                                        _mybir().ActivationFunctionType.Exp,
                                        scale=SCALE)
                                    # causal mask: multiplicative 0/1 on the
                                    # bf16 P tile, diagonal-strip slots only
                                    if ki >= 4 * qj:
                                        r = ki - 4 * qj
                                        cw = min(128 * (r + 1), 512)
                                        nc.vector.tensor_mul(
                                            pt[:, ci:cw], pt[:, ci:cw],
                                            mask_sb[:, r, ci:cw])
                                    # Z accumulation (DVE, bf16)
                                    if ki == 0:
                                        nc.vector.tensor_copy(zacc[:], pt[:])
                                    else:
                                        nc.vector.tensor_add(zacc[:, ci:],
                                                             zacc[:, ci:],
                                                             pt[:, ci:])
                                    pts[ki] = (pt, ci)

                            def emit_pv(kp, last, qj=qj, kv=kv, ps_o=ps_o,
                                        pts=pts, nkp=nkp):
                                for i in range(2):
                                    ki = 2 * kp + i
                                    pt, ci = pts.pop(ki)
                                    fst = (kp == 0 and i == 0)
                                    lst = last and i == 1
                                    nc.tensor.matmul(
                                        ps_o[:, ci:],
                                        vsb[:, ki, kv * 128:(kv + 1) * 128],
                                        pt[:, ci:],
                                        start=fst, stop=lst)

                            # exactly 4 o_proj pops per head so the previous
                            # strip's 32 tiles fully drain before its y-ring
                            # slot is rewritten (ring depth is 2)
                            npop = [0]
                            vpopped = [False]

                            def pop1(qj=qj, h=h):
                                # deferred V-proj tiles fill the otherwise
                                # ACT-paced first strip (timed so the xv DMA
                                # has landed by the first pop)
                                if (vproj_work and not vpopped[0]
                                        and qj == 0):
                                    emit_vproj(vproj_work.pop(0))
                                    vpopped[0] = True
                                    return
                                if npop[0] < 4:
                                    pop_oproj(1)
                                    npop[0] += 1

                            # spread the 4 o_proj pops evenly over the kp
                            # sites so deep strips keep PE filler through
                            # their back half too
                            quota = [0]

                            def pop_site(site, qj=qj):
                                quota[0] += 4
                                while quota[0] >= nkp and npop[0] < 4:
                                    quota[0] -= nkp
                                    pop1()

                            emit_scores(0)
                            if head_tail[0] is not None:
                                head_tail[0]()
                                head_tail[0] = None
                            pop_site(0)
                            for kp in range(1, nkp):
                                emit_scores(kp)
                                emit_pv(kp - 1, last=False)
                                pop_site(kp)
                            emit_pv(nkp - 1, last=True)
                            while npop[0] < 4:
                                pop1()

                            def tail(ps_o=ps_o, zacc=zacc, h=h, qj=qj,
                                     q0=q0):
                                # Z: exact fp32 cross-partition reduce on the
                                # (otherwise idle) gpsimd engine, then bf16
                                # reciprocal on DVE -- PE not involved
                                rbc = p2r.tile([128, 512], DT.bfloat16,
                                               tag="rbc")
                                nc.gpsimd.partition_all_reduce(
                                    rbc[:], zacc[:], 128, _bass_isa().ReduceOp.add)
                                with nc.allow_low_precision(
                                        reason="softmax recip in bf16"):
                                    nc.vector.reciprocal(rbc[:], rbc[:])
                                yt = p2y.tile([128, 512], DT.bfloat16,
                                              tag="yt")
                                nc.vector.tensor_mul(yt[:], ps_o[:], rbc[:])
                                rj = qj % 2
                                nc.scalar.copy(yh8[:, h, rj, :], yt[:])
                                nc.vector.tensor_sub(yl8[:, h, rj, :], yt[:],
                                                     yh8[:, h, rj, :])
                            if h == NHC - 1:
                                # last head of each strip: run the tail now,
                                # column-chunked, so the next block of o_proj
                                # work (which reads y per 128-column query
                                # tile) unblocks as early as possible. The
                                # final strip uses 4 chunks (its drain has
                                # nothing else to do); mid strips use 2 to
                                # halve the per-op overheads.
                                nch = 4 if qj == 3 else 2
                                cw = 512 // nch
                                rj = qj % 2
                                rbc = p2r.tile([128, 512], DT.bfloat16,
                                               tag="rbc")
                                yt = p2y.tile([128, 512], DT.bfloat16,
                                              tag="yt")
                                for c in range(nch):
                                    cs = slice(c * cw, (c + 1) * cw)
                                    nc.gpsimd.partition_all_reduce(
                                        rbc[:, cs], zacc[:, cs], 128,
                                        _bass_isa().ReduceOp.add)
                                    with nc.allow_low_precision(
                                            reason="softmax recip in bf16"):
                                        nc.vector.reciprocal(rbc[:, cs],
                                                             rbc[:, cs])
                                    nc.vector.tensor_mul(yt[:, cs],
                                                         ps_o[:, cs],
                                                         rbc[:, cs])
                                    nc.scalar.copy(yh8[:, h, rj, cs],
                                                   yt[:, cs])
                                    nc.vector.tensor_sub(yl8[:, h, rj, cs],
                                                         yt[:, cs],
                                                         yh8[:, h, rj, cs])
                            else:
                                head_tail[0] = tail
                        if head_tail[0] is not None:
                            head_tail[0]()
                            head_tail[0] = None
                        for tt in range(4 * qj, 4 * qj + 4):
                            for nt in range(8):
                                oproj_work.append((qj, tt, nt))
                    while oproj_work:
                        pop_oproj(1, use_ps_s=True)
    nc.compile()
    return nc


def _mybir():
    import concourse.mybir as mybir
    return mybir


def _bass_isa():
    import concourse.bass_isa as bass_isa
    return bass_isa


def _get_nc():
    if "nc" not in _CACHE:
        _CACHE["nc"] = _build_nc()
    return _CACHE["nc"]


def _split_w(wT):
    """Host hi/lo split of a transposed weight block, x32 pre-scale."""
    ws = np.asarray(wT, np.float32) * WS
    wh = ws.astype(FP8)
    wl = (ws - wh.astype(np.float32)).astype(FP8)
    return wh, wl


def _host_prep(hidden_states, position_ids, wq, wk, wv, wo):
    inv_freq = 1.0 / (ROPE_THETA ** (np.arange(0, D, 2, dtype=np.float32) / D))
    masks = np.zeros((128, 4, 512), dtype=np.float32)
    p = np.arange(128)[:, None]
    fidx = np.arange(512)[None, :]
    for r in range(4):
        masks[:, r, :] = np.where(128 * r + p > fidx, 0.0, 1.0)
    masks = masks.astype(BF16)

    hs = np.asarray(hidden_states, dtype=np.float32)
    x8, tabs = {}, {}
    for b in range(DP):
        xT = np.ascontiguousarray(hs[b].T)
        xh = xT.astype(FP8)
        xl = (xT - xh.astype(np.float32)).astype(FP8)
        x8[b] = (xh, xl)
        pos = np.asarray(position_ids[b], dtype=np.float32)
        freqs = pos[:, None] * inv_freq[None, :]
        emb = np.concatenate([freqs, freqs], axis=1)
        cos = (np.cos(emb).T / WS).astype(BF16)
        sin = np.sin(emb).T / WS
        # rows 0:64 = +sin (multiplies q[0:64] -> hi half), rows 64:128 = -sin
        # (multiplies q[64:128] -> lo half)
        sin2 = np.concatenate([sin[:64], -sin[:64]], axis=0).astype(BF16)
        tabs[b] = (np.ascontiguousarray(cos), np.ascontiguousarray(sin2))

    def _pack_kv(wT):
        # [H, DKV] -> [128, 2, KO, DKV] (hi,lo) so one DMA moves both with
        # 8KB contiguous per partition
        h8, l8 = _split_w(wT)
        h8 = h8.reshape(KO, 128, DKV).transpose(1, 0, 2)
        l8 = l8.reshape(KO, 128, DKV).transpose(1, 0, 2)
        return np.ascontiguousarray(np.stack([h8, l8], axis=1))

    wq8, wk8, wv8, wo8 = {}, {}, {}, {}
    for g in range(TP):
        qh, ql = _split_w(wq[DQ * g:DQ * (g + 1), :].T)   # [H, DQ]
        # hi+lo per-head contiguous layout [128, NHC, 2, KO, 128]
        qh = qh.reshape(KO, 128, NHC, 128).transpose(1, 2, 0, 3)
        ql = ql.reshape(KO, 128, NHC, 128).transpose(1, 2, 0, 3)
        wq8[g] = np.ascontiguousarray(np.stack([qh, ql], axis=2))
        wk8[g] = _pack_kv(wk[DKV * g:DKV * (g + 1), :].T)
        wv8[g] = _pack_kv(wv[DKV * g:DKV * (g + 1), :].T)
        wo8[g] = _split_w(wo[:, DQ * g:DQ * (g + 1)].T)

    in_maps = []
    for c in range(8):
        b, g = divmod(c, TP)
        in_maps.append({
            "xh8": x8[b][0], "xl8": x8[b][1],
            "wq8t": wq8[g], "wk8t": wk8[g], "wv8t": wv8[g],
            "woh8": wo8[g][0], "wol8": wo8[g][1],
            "cosb": tabs[b][0], "sin2b": tabs[b][1],
            "masks": masks,
        })
    return in_maps


def kernel(hidden_states, position_ids, wq, wk, wv, wo):
    from concourse.bass_utils import run_bass_kernel_spmd
    in_maps = _host_prep(np.asarray(hidden_states), np.asarray(position_ids),
                         np.asarray(wq), np.asarray(wk), np.asarray(wv),
                         np.asarray(wo))
    nc = _get_nc()
    res = run_bass_kernel_spmd(nc, in_maps, list(range(8)))
    out = np.zeros((DP, S, H), dtype=np.float32)
    for c in range(8):
        b = c // TP
        out[b] += res.results[c]["outp"].astype(np.float32)
    out *= 1.0 / WS
    return out

